# revision 1
# baseline (speedup 1.0000x reference)
"""Joint-entropy (KDE logsumexp over 3x3 windows) Trainium2 kernel.

Math: for each 3x3 window of pixel vectors v_n (C=3 channels),
  out[i,j] = log_norm - (1/9) * sum_n log(S_n),  S_n = sum_m exp(-2*||v_n-v_m||^2)
with log_norm = log(9) + 3*log(sqrt(2*pi)*0.5)  (h = 0.5, logits = -2*d2).

Sharding: 8 cores = 4 batches x 2 row-halves. Each core gets a host-padded
x[b,:,r0:r0+129,:] slice and produces a [127, 254] output slab. All window
math is local (halo rows included in the input slice), so no collectives.

On-chip layout: partitions = window rows (127), free dim = image cols
(padded by 2 on both sides so column-shifted reads stay in-bounds).
All row shifts are realized by loading 3 row-shifted copies of the input
(X[c,s][p,w] = x[c, p+s, w]); every other access is a free-dim (column)
shift, so no partition-shifted operands are needed anywhere.

E-map classes (s = window row of the anchor pixel, a = row gap, b = col gap):
  (s,0) s=0,1,2 with b in {1,2};  (s,1) s=0,1 with b in {-2..2};  (0,2) b in {-2..2}.
Each class is one [127, nb, 256] tile of exp(-2*d2) values, computed with
"wide" ops covering all nb column-gaps at once (stride-0 broadcast on the
anchor operand). S_n sums then read these at column offsets 0..2 only.
"""

import dataclasses

import numpy as np

import concourse.bacc as bacc
import concourse.bass as bass
import concourse.tile as tile
from concourse import mybir
from concourse.bass_utils import run_bass_kernel_spmd

F32 = mybir.dt.float32
BF16 = mybir.dt.bfloat16
AOP = mybir.AluOpType
AF = mybir.ActivationFunctionType

C = 3
W = 256
B = 4
R = 3
ROWS_IN = 129  # 127 window rows need input rows r0 .. r0+128
ROWS_OUT = 127
WOUT = 254
PAD = 2
WT = W + 2 * PAD  # padded width (host-padded)
LOG_NORM = float(np.log(9.0) + 3.0 * np.log(np.sqrt(2.0 * np.pi) * 0.5))

# (s, a, bmin, nb)
_CLASSES = [
    (0, 0, 1, 2),
    (1, 0, 1, 2),
    (2, 0, 1, 2),
    (0, 1, -2, 5),
    (1, 1, -2, 5),
    (0, 2, -2, 5),
]


def _role_terms():
    """For each window role (nr, nc) list the 8 cross terms as
    (s, a, plane_k, col_off): value = M[(s,a)][:, k, j + col_off]."""
    out = {}
    for nr in range(R):
        for ncol in range(R):
            tl = []
            for mc in range(R):  # same row, other columns
                if mc != ncol:
                    b = abs(mc - ncol)
                    tl.append((nr, 0, b - 1, min(ncol, mc)))
            for mr in range(R):  # other rows
                if mr == nr:
                    continue
                if mr > nr:
                    for mc in range(R):
                        tl.append((nr, mr - nr, mc - ncol + 2, ncol))
                else:
                    for mc in range(R):
                        tl.append((mr, nr - mr, ncol - mc + 2, mc))
            assert len(tl) == 8
            out[(nr, ncol)] = tl
    return out


def _wide_pair(xs_tile, xa_tile, bmin, nb):
    """APs for one wide sub: anchor broadcast over nb planes, other operand
    shifted by b = bmin..bmin+nb-1 columns per plane."""
    anchor = xs_tile[:, PAD : PAD + W].unsqueeze(1).to_broadcast([ROWS_OUT, nb, W])
    base = xa_tile[:, PAD + bmin : PAD + bmin + W].unsqueeze(1)
    shifted = dataclasses.replace(
        base, ap=[list(base.ap[0]), [1, nb], list(base.ap[2])]
    )
    return anchor, shifted


def _build_program():
    nc = bacc.Bacc("TRN2")
    xin = nc.dram_tensor("xin", (C, ROWS_IN, WT), F32, kind="ExternalInput")
    yout = nc.dram_tensor("yout", (ROWS_OUT, WOUT), F32, kind="ExternalOutput")

    with tile.TileContext(nc) as tc:
        with (
            tc.tile_pool(name="xp", bufs=1) as xp,
            tc.tile_pool(name="mp", bufs=1) as mp,
            tc.tile_pool(name="tp", bufs=2) as tp,
            tc.tile_pool(name="sp", bufs=1) as sp,
        ):
            # ---- stage A: row-shifted input copies (single DMA each) ------
            X = {}
            for c in range(C):
                for s in range(R):
                    t = xp.tile([ROWS_OUT, WT], F32, tag=f"x_{c}_{s}")
                    nc.gpsimd.dma_start(out=t, in_=xin[c, s : s + ROWS_OUT, :])
                    X[(c, s)] = t

            # ---- stage B: E maps, wide over column-gap planes -------------
            M = {}
            for s, a, bmin, nb in _CLASSES:
                mt = mp.tile([ROWS_OUT, nb, W], BF16, tag=f"m_{s}_{a}")
                sq = []
                for c in range(C):
                    d = tp.tile([ROWS_OUT, nb, W], F32, tag=f"d{c}")
                    a0, a1 = _wide_pair(X[(c, s)], X[(c, s + a)], bmin, nb)
                    nc.vector.tensor_sub(d, a0, a1)
                    q = tp.tile([ROWS_OUT, nb, W], F32, tag=f"q{c}")
                    nc.scalar.square(q, d)
                    sq.append(q)
                d2t = tp.tile([ROWS_OUT, nb, W], F32, tag="d2")
                nc.gpsimd.tensor_add(d2t, sq[0], sq[1])
                nc.gpsimd.tensor_add(d2t, d2t, sq[2])
                nc.scalar.activation(mt, d2t, AF.Exp, scale=-2.0)
                M[(s, a)] = mt

            # ---- stage C: per-role S sums ---------------------------------
            def term_ap(t4):
                s, a, k, c0 = t4
                return M[(s, a)][:, k, c0 : c0 + WOUT]

            S = []
            for role, tl in _role_terms().items():
                st = sp.tile([ROWS_OUT, WOUT], BF16, tag=f"s_{role[0]}_{role[1]}")
                nc.vector.scalar_tensor_tensor(
                    out=st,
                    in0=term_ap(tl[0]),
                    scalar=1.0,
                    in1=term_ap(tl[1]),
                    op0=AOP.add,
                    op1=AOP.add,
                )
                for t4 in tl[2:]:
                    nc.vector.tensor_add(st, st, term_ap(t4))
                S.append(st)

            # ---- stage D: product of 9 S maps, log, affine ----------------
            def mul(x, y, tag):
                o = sp.tile([ROWS_OUT, WOUT], BF16, tag=tag)
                nc.vector.tensor_mul(o, x, y)
                return o

            p01 = mul(S[0], S[1], "p01")
            p23 = mul(S[2], S[3], "p23")
            p45 = mul(S[4], S[5], "p45")
            p67 = mul(S[6], S[7], "p67")
            q0 = mul(p01, p23, "q0")
            q1 = mul(p45, p67, "q1")
            q2 = mul(q0, q1, "q2")
            P = mul(q2, S[8], "pp")

            L = sp.tile([ROWS_OUT, WOUT], F32, tag="ln")
            nc.scalar.activation(L, P, AF.Ln)
            OUT = sp.tile([ROWS_OUT, WOUT], F32, tag="out")
            nc.vector.tensor_scalar(
                out=OUT,
                in0=L,
                scalar1=-1.0 / 9.0,
                scalar2=LOG_NORM,
                op0=AOP.mult,
                op1=AOP.add,
            )
            nc.gpsimd.dma_start(out=yout[:, :], in_=OUT)
    if not nc.is_finalized():
        nc.finalize()
    return nc


_PROGRAM = None


def _get_program():
    global _PROGRAM
    if _PROGRAM is None:
        _PROGRAM = _build_program()
    return _PROGRAM


def _shard_inputs(x):
    x = np.asarray(x, dtype=np.float32)
    xp = np.zeros((B, C, 256, WT), dtype=np.float32)
    xp[:, :, :, PAD : PAD + W] = x
    in_maps = []
    for core in range(8):
        b, half = divmod(core, 2)
        r0 = half * 127
        in_maps.append({"xin": np.ascontiguousarray(xp[b, :, r0 : r0 + ROWS_IN, :])})
    return in_maps


def _gather(results):
    out = np.empty((B, 254, 254), dtype=np.float32)
    for core in range(8):
        b, half = divmod(core, 2)
        out[b, half * 127 : half * 127 + 127, :] = results[core]["yout"]
    return out


def kernel(x, **_unused):
    nc = _get_program()
    res = run_bass_kernel_spmd(nc, _shard_inputs(x), core_ids=list(range(8)))
    return _gather(res.results)


def kernel_traced(x):
    """Same as kernel() but returns (output, BassKernelResults) with trace."""
    nc = _get_program()
    res = run_bass_kernel_spmd(
        nc, _shard_inputs(x), core_ids=list(range(8)), trace=True
    )
    return _gather(res.results), res



# revision 5
# speedup vs baseline: 1.2609x; 1.2609x over previous
"""Joint-entropy (KDE logsumexp over 3x3 windows) Trainium2 kernel, v2.

Math: for each 3x3 window of pixel vectors v_n (C=3 channels),
  out[i,j] = log_norm - (1/9) * sum_n log(S_n),  S_n = sum_m exp(-2*||v_n-v_m||^2)
with log_norm = log(9) + 3*log(sqrt(2*pi)*0.5)  (h = 0.5, logits = -2*d2).

Sharding: 8 cores = 4 batches x 2 row-halves. Each core gets a host-padded
bf16 x[b, r0:r0+129, :, :] slab ([129, 3, 264], row-major) and produces a
[127, 254] f32 output slab. No collectives.

On-chip layout: partitions = image rows. Two row-shifted input tiles
X0 = rows 0..127, X1 = rows 1..128 (two DMA reads of the same DRAM slab)
make every row-gap reachable with partition offsets 0/+1 only.

E maps (exp(-2*d2) per pixel-pair offset (a,b)), deduplicated:
  M1  [128,5,260]: a=1, b=-2..2, anchor rows 0..127   (X0 vs X1)
  M2  [127,5,260]: a=2, b=-2..2, anchor rows 0..126   (X0 vs X1[+1])
  E0lo[128,2,260]: a=0, b=1,2,   anchor rows 0..127   (X0 vs X0)
  E0hi[128,2,260]: a=0, b=1,2,   anchor rows 1..128   (X1 vs X1)
Each class is computed with channel-fused wide ops ([P, nb, 3, 260] subs /
squares, [P, nb, 260] channel-sum adds, one Exp).  Anchor cols span -2..257
(tile col u = image col u-2) so that up-direction combo reads stay in range;
input is host-padded to width 264 (4 zeros each side).

Stage C: per-(a, direction) 3-term sliding sums over the b axis (2 wide adds
each via plane-stride tricks), then the 9 role sums = 3 wide adds pairs over
[127, 3, 254] with per-plane column offsets encoded in the plane stride.
The "+1" self term is folded into the same-row A combos.

Stage D: product of the 9 S maps (4 muls), one Ln, one affine.
"""

import dataclasses

import numpy as np
import ml_dtypes

import concourse.bacc as bacc
import concourse.tile as tile
from concourse import mybir
from concourse.bass_utils import run_bass_kernel_spmd

F32 = mybir.dt.float32
BF16 = mybir.dt.bfloat16
AOP = mybir.AluOpType
AF = mybir.ActivationFunctionType

B = 4
C = 3
W = 256
PAD = 4          # host zero-pad on each side
WT = W + 2 * PAD  # 264
WA = W + 4       # 260: anchor cols -2..257 (tile col u = image col u-2)
ROWS_IN = 129
P = 128
POUT = 127
WOUT = 254
LOG_NORM = float(np.log(9.0) + 3.0 * np.log(np.sqrt(2.0 * np.pi) * 0.5))


def _shift_planes(base_ap, dplane, nplanes):
    """Rewrite the (unsqueezed) plane dim of `base_ap` to [dplane, nplanes].

    dplane is in elements: plane_stride +/- k encodes a per-plane column
    shift of -/+ k relative to the tile's natural plane stride."""
    ap = [list(d) for d in base_ap.ap]
    ap[1] = [dplane, nplanes]
    return dataclasses.replace(base_ap, ap=ap)


def _build_program():
    nc = bacc.Bacc("TRN2")
    xin = nc.dram_tensor("xin", (ROWS_IN, C, WT), BF16, kind="ExternalInput")
    yout = nc.dram_tensor("yout", (POUT, WOUT), F32, kind="ExternalOutput")

    with tile.TileContext(nc) as tc:
        with tc.tile_pool(name="p", bufs=1) as tp:
            # ---- input: three row-shifted views of the same DRAM slab ---
            # (compute-engine operands must start at partition 0, so every
            # row gap needs its own partition-0-aligned copy)
            X0 = tp.tile([P, C, WT], BF16, tag="x0")
            X1 = tp.tile([P, C, WT], BF16, tag="x1")
            X2 = tp.tile([POUT, C, WT], BF16, tag="x2")
            nc.sync.dma_start(out=X0, in_=xin[0:P, :, :])
            nc.sync.dma_start(out=X1, in_=xin[1 : 1 + P, :, :])
            nc.sync.dma_start(out=X2, in_=xin[2 : 2 + POUT, :, :])

            # ---- stage B: E-map classes ---------------------------------
            # (tag, rows, anchor_tile, anchor_p0, other_tile, other_p0,
            #  col0_other, nb)   with plane k of `other` at col col0+k.
            classes = [
                ("m1", P, X0, 0, X1, 0, 0, 5),      # a=1, b=-2..2
                ("m2", POUT, X0, 0, X2, 0, 0, 5),   # a=2, b=-2..2
                ("e0lo", P, X0, 0, X0, 0, 3, 2),    # a=0, b=1,2
                ("e0hi", P, X1, 0, X1, 0, 3, 2),    # a=0, b=1,2
            ]
            M = {}
            for tag, rows, xa, pa, xb, pb, c0, nb in classes:
                anchor = (
                    xa[pa : pa + rows, :, 2 : 2 + WA]
                    .unsqueeze(1)
                    .to_broadcast([rows, nb, C, WA])
                )
                other = _shift_planes(
                    xb[pb : pb + rows, :, c0 : c0 + WA].unsqueeze(1), 1, nb
                )
                d = tp.tile([rows, nb, C, WA], BF16, tag=f"d_{tag}")
                nc.vector.tensor_sub(d, anchor, other)
                q = tp.tile([rows, nb, C, WA], BF16, tag=f"q_{tag}")
                if tag == "m2":
                    nc.scalar.activation(q, d, AF.Square)
                else:
                    nc.vector.tensor_mul(q, d, d)
                d2 = tp.tile([rows, nb, WA], BF16, tag=f"d2_{tag}")
                eng = nc.gpsimd if tag in ("e0lo", "e0hi") else nc.vector
                s01 = tp.tile([rows, nb, WA], BF16, tag=f"s01_{tag}")
                eng.tensor_add(s01, q[:, :, 0, :], q[:, :, 1, :])
                eng.tensor_add(d2, s01, q[:, :, 2, :])
                m = tp.tile([rows, nb, WA], BF16, tag=f"m_{tag}")
                nc.scalar.activation(m, d2, AF.Exp, scale=-2.0)
                M[tag] = m

            # ---- stage C: combos ----------------------------------------
            # D combos: plane t = sum of M planes t..t+2 (col +0 each);
            # read later at plane t=2-nc.  Tile cols = image cols 0..255.
            # U combos: plane t(=nc) = sum over k=t..t+2 of M plane k at
            # image col j read from M tile col j+4-k.
            SM = WA  # plane stride (elements) of M tiles

            def d_combo(mt, rows, tag):
                t4 = tp.tile([rows, 4, W], BF16, tag=f"t4{tag}")
                nc.vector.tensor_add(
                    t4, mt[0:rows, 0:4, 2 : 2 + W], mt[0:rows, 1:5, 2 : 2 + W]
                )
                out = tp.tile([rows, 3, W], BF16, tag=f"dc{tag}")
                nc.vector.tensor_add(
                    out, t4[0:rows, 0:3, :], mt[0:rows, 2:5, 2 : 2 + W]
                )
                return out

            def u_combo(mt, rows, tag):
                t4 = tp.tile([rows, 4, W], BF16, tag=f"u4{tag}")
                in0 = _shift_planes(mt[0:rows, 0:1, 4 : 4 + W], SM - 1, 4)
                in1 = _shift_planes(mt[0:rows, 1:2, 3 : 3 + W], SM - 1, 4)
                nc.vector.tensor_add(t4, in0, in1)
                out = tp.tile([rows, 3, W], BF16, tag=f"uc{tag}")
                in2 = _shift_planes(mt[0:rows, 2:3, 2 : 2 + W], SM - 1, 3)
                nc.vector.tensor_add(out, t4[0:rows, 0:3, :], in2)
                return out

            D1 = d_combo(M["m1"], P, "d1")
            U1 = u_combo(M["m1"], P, "u1")
            D2 = d_combo(M["m2"], POUT, "d2")
            U2 = u_combo(M["m2"], POUT, "u2")

            # A combos (same-row pair sums, +1 self term folded in):
            #  plane nc=0: E01(j)   + E02(j);    nc=1: E01(j-1) + E01(j)
            #  plane nc=2: E02(j-2) + E01(j-1)   (tile col = image col + 2)
            def a_combo(et, tag):
                out = tp.tile([P, 3, W], BF16, tag=f"a{tag}")
                pairs = [
                    ((0, 2), (1, 2)),
                    ((0, 1), (0, 2)),
                    ((1, 0), (0, 1)),
                ]
                for ncol, ((k0, o0), (k1, o1)) in enumerate(pairs):
                    nc.vector.scalar_tensor_tensor(
                        out=out[:, ncol, :],
                        in0=et[0:P, k0, o0 : o0 + W],
                        scalar=1.0,
                        in1=et[0:P, k1, o1 : o1 + W],
                        op0=AOP.add,
                        op1=AOP.add,
                    )
                return out

            Alo = a_combo(M["e0lo"], "lo")
            Ahi = a_combo(M["e0hi"], "hi")

            # Partition-shifted copies (compute operands can't start at a
            # nonzero partition; DMA can move data across partitions).
            def pshift(t, tag):
                o = tp.tile([POUT, 3, W], BF16, tag=f"ps{tag}")
                nc.sync.dma_start(out=o, in_=t[1 : 1 + POUT, :, :])
                return o

            D1h = pshift(D1, "d1")
            U1h = pshift(U1, "u1")
            Ahh = pshift(Ahi, "ah")

            # ---- role sums: S_nr [127, 3(nc), 254] ----------------------
            SA = W  # plane stride of A/D/U combo tiles

            def diag(t, plane0, dplane, col0):
                """[POUT, 3, WOUT] view of t: plane i at (plane0 + i*dplane,
                col col0 + i)."""
                base = t[0:POUT, plane0 : plane0 + 1, col0 : col0 + WOUT]
                return _shift_planes(base, dplane * SA + 1, 3)

            def role(a_ap, b_ap, c_ap, tag):
                s = tp.tile([POUT, 3, WOUT], BF16, tag=f"s{tag}")
                nc.vector.tensor_add(s, a_ap, b_ap)
                nc.vector.tensor_add(s, s, c_ap)
                return s

            S0 = role(diag(Alo, 0, 1, 0), diag(D1, 2, -1, 0),
                      diag(D2, 2, -1, 0), "0")
            S1 = role(diag(Ahi, 0, 1, 0), diag(U1, 0, 1, 0),
                      diag(D1h, 2, -1, 0), "1")
            S2 = role(diag(Ahh, 0, 1, 0), diag(U1h, 0, 1, 0),
                      diag(U2, 0, 1, 0), "2")

            # ---- stage D: product, log, affine --------------------------
            T0 = tp.tile([POUT, 3, WOUT], BF16, tag="t0")
            nc.vector.tensor_mul(T0, S0, S1)
            T1 = tp.tile([POUT, 3, WOUT], BF16, tag="t1")
            nc.vector.tensor_mul(T1, T0, S2)
            R = tp.tile([POUT, WOUT], BF16, tag="r")
            nc.vector.tensor_mul(R, T1[:, 0, :], T1[:, 1, :])
            PP = tp.tile([POUT, WOUT], BF16, tag="pp")
            nc.vector.tensor_mul(PP, R, T1[:, 2, :])
            L = tp.tile([POUT, WOUT], F32, tag="ln")
            nc.scalar.activation(L, PP, AF.Ln)
            OUT = tp.tile([POUT, WOUT], F32, tag="out")
            nc.vector.tensor_scalar(
                out=OUT,
                in0=L,
                scalar1=-1.0 / 9.0,
                scalar2=LOG_NORM,
                op0=AOP.mult,
                op1=AOP.add,
            )
            nc.sync.dma_start(out=yout[:, :], in_=OUT)
    if not nc.is_finalized():
        nc.finalize()
    return nc


_PROGRAM = None


def _get_program():
    global _PROGRAM
    if _PROGRAM is None:
        _PROGRAM = _build_program()
    return _PROGRAM


def _shard_inputs(x):
    x = np.asarray(x, dtype=np.float32)
    xp = np.zeros((B, 256, C, WT), dtype=ml_dtypes.bfloat16)
    xp[:, :, :, PAD : PAD + W] = np.transpose(x, (0, 2, 1, 3))
    in_maps = []
    for core in range(8):
        b, half = divmod(core, 2)
        r0 = half * POUT
        in_maps.append({"xin": np.ascontiguousarray(xp[b, r0 : r0 + ROWS_IN])})
    return in_maps


def _gather(results):
    out = np.empty((B, 254, 254), dtype=np.float32)
    for core in range(8):
        b, half = divmod(core, 2)
        out[b, half * POUT : half * POUT + POUT, :] = results[core]["yout"]
    return out


def kernel(x, **_unused):
    nc = _get_program()
    res = run_bass_kernel_spmd(nc, _shard_inputs(x), core_ids=list(range(8)))
    return _gather(res.results)


def kernel_traced(x):
    """Same as kernel() but returns (output, BassKernelResults) with trace."""
    nc = _get_program()
    res = run_bass_kernel_spmd(
        nc, _shard_inputs(x), core_ids=list(range(8)), trace=True
    )
    return _gather(res.results), res


# revision 7
# speedup vs baseline: 1.6886x; 1.3392x over previous
"""Joint-entropy (KDE logsumexp over 3x3 windows) Trainium2 kernel, v3.

Math: for each 3x3 window of pixel vectors v_n (C=3 channels),
  out[i,j] = log_norm - (1/9) * sum_n log(S_n),  S_n = sum_m exp(-2*||v_n-v_m||^2)
with log_norm = log(9) + 3*log(sqrt(2*pi)*0.5)  (h = 0.5, logits = -2*d2).

Per-pair Gaussian via the Act engine's Derivative_Erf:
  derf(sqrt(2)*d_c) = (2/sqrt(pi)) * exp(-2*d_c^2)
so  prod_c derf(...) = k * exp(-2*||d||^2),  k = (2/sqrt(pi))^3.
Every E value carries the factor k; the self-term "+1" becomes "+k"
(folded into the A combos) and the final affine subtracts ln(k):
  P' = prod_n (k + sum_m!=n kE) = k^9 * prod_n S_n
  out = (log_norm + ln k) - (1/9) * ln P'.

Sharding: 8 cores = 4 batches x 2 row-halves; host-padded bf16 slab
[130, 3, 264] per core ([129 real rows + 1 pad], channels mid, width
padded 4 each side); output [127, 254] f32. No collectives.

On-chip: partitions = image rows. One fused DMA loads three row-shifted
views X[:, s] = rows s..s+127 (s = 0,1,2) so every row gap is reachable
with partition-0-aligned operands. E-map classes (sub -> derf -> 2 chmuls):
  m1  [128,5,260] a=1 (X0 vs X1),  m2 [127,5,260] a=2 (X0 vs X2),
  e0lo[128,2,260] a=0 rows 0..127, e0hi[128,2,260] a=0 rows 1..128.
Anchor cols span -2..257 (tile col u = image col u-2).

Stage C: sliding 3-sums over the b axis (D = down pairs, U = up pairs with
per-plane column shifts encoded in the plane stride), A = same-row pairs
(+k folded). Partition-shifted copies D1h/U1h/Ahh via SBUF-to-SBUF DMA
(compute operands must start at partition 0; DMA can shift partitions).
Role sums = 3 wide add-pairs over [127, 3(nc), 254] using diagonal
plane-stride reads. Stage D: 4 muls, Ln, affine.

DMAs ride the gpsimd software-DGE queue (descriptors fan out over all 16
DMA engines; a HWDGE queue pins a single ~22 GB/s engine).
"""

import dataclasses

import numpy as np
import ml_dtypes

import concourse.bacc as bacc
import concourse.tile as tile
from concourse import mybir
from concourse.bass_utils import run_bass_kernel_spmd

F32 = mybir.dt.float32
BF16 = mybir.dt.bfloat16
AOP = mybir.AluOpType
AF = mybir.ActivationFunctionType

B = 4
C = 3
W = 256
PAD = 4           # host zero-pad each side
WT = W + 2 * PAD  # 264
WA = W + 4        # 260: anchor cols -2..257
ROWS_IN = 129
P = 128
POUT = 127
WOUT = 254
SQRT2 = float(np.sqrt(2.0))
CDERF = float(2.0 / np.sqrt(np.pi))   # Derivative_Erf scale constant
K = CDERF**3                          # factor carried by every E value
LOG_NORM = float(np.log(9.0) + 3.0 * np.log(np.sqrt(2.0 * np.pi) * 0.5))
AFFINE_C = LOG_NORM + 3.0 * float(np.log(CDERF))  # + (1/9)*ln(k^9)/... = ln k


def _shift_planes(base_ap, dplane, nplanes):
    """Rewrite the (unsqueezed) plane dim of `base_ap` to [dplane, nplanes].

    dplane is in elements; plane_stride +/- j vs the natural stride encodes a
    per-plane column shift."""
    ap = [list(d) for d in base_ap.ap]
    ap[1] = [dplane, nplanes]
    return dataclasses.replace(base_ap, ap=ap)


def _build_program():
    nc = bacc.Bacc("TRN2")
    xin = nc.dram_tensor("xin", (ROWS_IN + 1, C, WT), BF16, kind="ExternalInput")
    yout = nc.dram_tensor("yout", (POUT, WOUT), F32, kind="ExternalOutput")

    with tile.TileContext(nc) as tc:
        with tc.tile_pool(name="p", bufs=1) as tp:
            # ---- one fused load: X[p, s] = input row p+s ----------------
            X = tp.tile([P, 3, C, WT], BF16, tag="x")
            src = _shift_planes(xin[0:P, :, :].unsqueeze(1), C * WT, 3)
            nc.gpsimd.dma_start(out=X, in_=src)

            # ---- stage B: E-map classes (sub -> derf -> chmuls) ---------
            # (tag, rows, anchor_s, other_s, col0_other, nb); plane j of
            # `other` reads col col0+j  (b = col0+j-2 relative to anchor).
            classes = [
                ("m1", P, 0, 1, 0, 5),
                ("m2", POUT, 0, 2, 0, 5),
                ("e0hi", P, 1, 1, 3, 2),
                ("e0lo", P, 0, 0, 3, 2),
            ]
            M = {}
            for tag, rows, sa, sb, c0, nb in classes:
                anchor = (
                    X[0:rows, sa, :, 2 : 2 + WA]
                    .unsqueeze(1)
                    .to_broadcast([rows, nb, C, WA])
                )
                other = _shift_planes(
                    X[0:rows, sb, :, c0 : c0 + WA].unsqueeze(1), 1, nb
                )
                d = tp.tile([rows, nb, C, WA], BF16, tag=f"d_{tag}")
                nc.vector.tensor_sub(d, anchor, other)
                g = tp.tile([rows, nb, C, WA], BF16, tag=f"g_{tag}")
                nc.scalar.activation(g, d, AF.Derivative_Erf, scale=SQRT2)
                g01 = tp.tile([rows, nb, WA], BF16, tag=f"g01_{tag}")
                nc.vector.tensor_mul(g01, g[:, :, 0, :], g[:, :, 1, :])
                m = tp.tile([rows, nb, WA], BF16, tag=f"m_{tag}")
                nc.vector.tensor_mul(m, g01, g[:, :, 2, :])
                M[tag] = m

            # ---- stage C: combos ----------------------------------------
            SM = WA  # plane stride (elements) of M tiles

            def d_combo(mt, rows, tag):
                t4 = tp.tile([rows, 4, W], BF16, tag=f"t4{tag}")
                nc.vector.tensor_add(
                    t4, mt[0:rows, 0:4, 2 : 2 + W], mt[0:rows, 1:5, 2 : 2 + W]
                )
                out = tp.tile([rows, 3, W], BF16, tag=f"dc{tag}")
                nc.vector.tensor_add(
                    out, t4[0:rows, 0:3, :], mt[0:rows, 2:5, 2 : 2 + W]
                )
                return out

            def u_combo(mt, rows, tag):
                # plane t = sum_{j=t..t+2} mt[:, j, col + 4 - j]
                t4 = tp.tile([rows, 4, W], BF16, tag=f"u4{tag}")
                in0 = _shift_planes(mt[0:rows, 0:1, 4 : 4 + W], SM - 1, 4)
                in1 = _shift_planes(mt[0:rows, 1:2, 3 : 3 + W], SM - 1, 4)
                nc.vector.tensor_add(t4, in0, in1)
                out = tp.tile([rows, 3, W], BF16, tag=f"uc{tag}")
                in2 = _shift_planes(mt[0:rows, 2:3, 2 : 2 + W], SM - 1, 3)
                nc.vector.tensor_add(out, t4[0:rows, 0:3, :], in2)
                return out

            # A combos (same-row pair sums, +k self term folded in):
            #  nc=0: E01(j)   + E02(j);   nc=1: E01(j-1) + E01(j)
            #  nc=2: E02(j-2) + E01(j-1)  (tile col = image col + 2)
            def a_combo(et, tag):
                out = tp.tile([P, 3, W], BF16, tag=f"a{tag}")
                pairs = [
                    ((0, 2), (1, 2)),
                    ((0, 1), (0, 2)),
                    ((1, 0), (0, 1)),
                ]
                for ncol, ((k0, o0), (k1, o1)) in enumerate(pairs):
                    nc.vector.scalar_tensor_tensor(
                        out=out[:, ncol, :],
                        in0=et[0:P, k0, o0 : o0 + W],
                        scalar=K,
                        in1=et[0:P, k1, o1 : o1 + W],
                        op0=AOP.add,
                        op1=AOP.add,
                    )
                return out

            D1 = d_combo(M["m1"], P, "d1")
            U1 = u_combo(M["m1"], P, "u1")
            Ahi = a_combo(M["e0hi"], "hi")

            # partition-shifted copies (rows +1) via SBUF-to-SBUF DMA
            def pshift(t, tag):
                o = tp.tile([POUT, 3, W], BF16, tag=f"ps{tag}")
                nc.gpsimd.dma_start(out=o, in_=t[1 : 1 + POUT, :, :])
                return o

            D1h = pshift(D1, "d1")
            U1h = pshift(U1, "u1")
            Ahh = pshift(Ahi, "ah")

            D2 = d_combo(M["m2"], POUT, "d2")
            U2 = u_combo(M["m2"], POUT, "u2")
            Alo = a_combo(M["e0lo"], "lo")

            # ---- role sums: S_nr [127, 3(nc), 254] ----------------------
            SA = W

            def diag(t, plane0, dplane, col0=0):
                base = t[0:POUT, plane0 : plane0 + 1, col0 : col0 + WOUT]
                return _shift_planes(base, dplane * SA + 1, 3)

            def role(a_ap, b_ap, c_ap, tag):
                s = tp.tile([POUT, 3, WOUT], BF16, tag=f"s{tag}")
                nc.vector.tensor_add(s, a_ap, b_ap)
                nc.vector.tensor_add(s, s, c_ap)
                return s

            S0 = role(diag(Alo, 0, 1), diag(D1, 2, -1), diag(D2, 2, -1), "0")
            S1 = role(diag(Ahi, 0, 1), diag(U1, 0, 1), diag(D1h, 2, -1), "1")
            S2 = role(diag(Ahh, 0, 1), diag(U1h, 0, 1), diag(U2, 0, 1), "2")

            # ---- stage D: product, log, affine --------------------------
            T0 = tp.tile([POUT, 3, WOUT], BF16, tag="t0")
            nc.vector.tensor_mul(T0, S0, S1)
            T1 = tp.tile([POUT, 3, WOUT], BF16, tag="t1")
            nc.vector.tensor_mul(T1, T0, S2)
            R = tp.tile([POUT, WOUT], BF16, tag="r")
            nc.vector.tensor_mul(R, T1[:, 0, :], T1[:, 1, :])
            PP = tp.tile([POUT, WOUT], BF16, tag="pp")
            nc.vector.tensor_mul(PP, R, T1[:, 2, :])
            L = tp.tile([POUT, WOUT], F32, tag="ln")
            nc.scalar.activation(L, PP, AF.Ln)
            OUT = tp.tile([POUT, WOUT], F32, tag="out")
            nc.vector.tensor_scalar(
                out=OUT,
                in0=L,
                scalar1=-1.0 / 9.0,
                scalar2=AFFINE_C,
                op0=AOP.mult,
                op1=AOP.add,
            )
            nc.gpsimd.dma_start(out=yout[:, :], in_=OUT)
    if not nc.is_finalized():
        nc.finalize()
    return nc


_PROGRAM = None


def _get_program():
    global _PROGRAM
    if _PROGRAM is None:
        _PROGRAM = _build_program()
    return _PROGRAM


def _shard_inputs(x):
    x = np.asarray(x, dtype=np.float32)
    xp = np.zeros((B, 257, C, WT), dtype=ml_dtypes.bfloat16)
    xp[:, :256, :, PAD : PAD + W] = np.transpose(x, (0, 2, 1, 3))
    in_maps = []
    for core in range(8):
        b, half = divmod(core, 2)
        r0 = half * POUT
        in_maps.append({"xin": np.ascontiguousarray(xp[b, r0 : r0 + ROWS_IN + 1])})
    return in_maps


def _gather(results):
    out = np.empty((B, 254, 254), dtype=np.float32)
    for core in range(8):
        b, half = divmod(core, 2)
        out[b, half * POUT : half * POUT + POUT, :] = results[core]["yout"]
    return out


def kernel(x, **_unused):
    nc = _get_program()
    res = run_bass_kernel_spmd(nc, _shard_inputs(x), core_ids=list(range(8)))
    return _gather(res.results)


def kernel_traced(x):
    """Same as kernel() but returns (output, BassKernelResults) with trace."""
    nc = _get_program()
    res = run_bass_kernel_spmd(
        nc, _shard_inputs(x), core_ids=list(range(8)), trace=True
    )
    return _gather(res.results), res


# revision 14
# speedup vs baseline: 1.8705x; 1.1077x over previous
"""Joint-entropy (KDE logsumexp over 3x3 windows) Trainium2 kernel, v4.

Math: for each 3x3 window of pixel vectors v_n (C=3 channels),
  out[i,j] = log_norm - (1/9) * sum_n log(S_n),  S_n = sum_m exp(-2*||v_n-v_m||^2)

Per-pair Gaussians via Act's Derivative_Erf: derf(sqrt(2)*d) =
(2/sqrt(pi))*exp(-2 d^2), so prod_c derf = k*exp(-2||d||^2), k=(2/sqrt(pi))^3.
Every E value carries k; the self term "+1" becomes "+k" (folded into the
same-row A combos) and the final affine adds ln(k):
  out = (log_norm + ln k) - (1/9) * ln prod_n (k + sum_{m!=n} kE_nm).

Sharding: 8 cores = 4 batches x 2 row-halves; host-prepped bf16 slab
[130, 3, 264] per core (129 rows + 1 pad row, width padded 4 each side);
output [127, 254] f32. partitions = image rows; X[p, s] = row p+s (one
fused DMA, contiguous 4.6KB descriptors) makes all row gaps reachable with
partition-0-aligned operands.

E classes, merged across the s axis into wide 4-free-dim ops:
  m12 [128, 2(a=1,2), 5(b), 3, 260]  (X0 vs X1 / X0 vs X2; a=2 row 127 is
      junk from the pad row, never consumed)
  e0  [128, 2(lo,hi), 2(b=1,2), 3, 260]  (X0 vs X0 rows 0..127 /
      X1 vs X1 rows 1..128)
Pipeline per class: sub (DVE) -> derf (Act) -> 2 channel-muls (DVE).

Stage C: sliding 3-sums over b (D = down pairs at col +0, U = up pairs with
per-plane col shifts via plane-stride tricks), A combos with +k folded.
Y = Ahi + U1 lets roles S1 = Y + D1h and S2 = Y@+1 + U2, so only TWO
partition-shifted copies (D1h, Yh) are needed, via SBUF-to-SBUF DMA
(compute operands must start at partition 0; DMA can cross partitions).
Stage D: 4 muls, Ln, affine. Output DMA is split over the three HWDGE
queues (SP/Act/DVE) to avoid the long gpsimd SWDGE drain at exit.
"""

import dataclasses

import numpy as np
import ml_dtypes

import concourse.bacc as bacc
import concourse.tile as tile
from concourse import mybir
from concourse.bass_utils import run_bass_kernel_spmd

F32 = mybir.dt.float32
BF16 = mybir.dt.bfloat16
AOP = mybir.AluOpType
AF = mybir.ActivationFunctionType

B = 4
C = 3
W = 256
PAD = 4           # host zero-pad each side
WT = W + 2 * PAD  # 264
WA = W + 4        # 260: anchor cols -2..257
ROWS_IN = 129
P = 128
POUT = 127
WOUT = 254
SROW = C * WT     # one input row in elements (792)
SQRT2 = float(np.sqrt(2.0))
CDERF = float(2.0 / np.sqrt(np.pi))
K = CDERF**3
LOG_NORM = float(np.log(9.0) + 3.0 * np.log(np.sqrt(2.0 * np.pi) * 0.5))
AFFINE_C = LOG_NORM + 3.0 * float(np.log(CDERF))


def _with_dims(base_ap, dims):
    """Replace free dims 1.. of `base_ap` (partition dim kept) with the given
    [stride, count] pairs (strides in elements)."""
    ap = [list(base_ap.ap[0])] + [list(d) for d in dims]
    return dataclasses.replace(base_ap, ap=ap)


def _build_program():
    nc = bacc.Bacc("TRN2")
    xin = nc.dram_tensor("xin", (ROWS_IN + 1, C, WT), BF16, kind="ExternalInput")
    yout = nc.dram_tensor("yout", (POUT, WOUT), F32, kind="ExternalOutput")

    with tile.TileContext(nc) as tc:
        with tc.tile_pool(name="p", bufs=1) as tp:
            # ---- one fused load: X[p, s] = input row p+s ----------------
            X = tp.tile([P, 3, C, WT], BF16, tag="x")
            src = _with_dims(xin[0:P, :, :], [[SROW, 3], [WT, C], [1, WT]])
            nc.gpsimd.dma_start(out=X, in_=src)

            def ap_of(base, elem_off, dims):
                """AP from `base` (a partition-sliced AP) with free dims
                replaced by `dims` and offset bumped by elem_off elements."""
                return dataclasses.replace(
                    _with_dims(base, dims), offset=base.offset + elem_off
                )

            # merged M tiles: chmuls of each class write one s-slice
            M12 = tp.tile([P, 2, 5, WA], BF16, tag="m12")  # s: a=1, a=2
            ME = tp.tile([P, 2, 2, WA], BF16, tag="me")    # s: lo, hi

            def cls(tag, rows, s_a, s_b, nb, c0, mt, s_out):
                """One E class (compute ops max out at 3 free dims):
                anchor X[:, s_a] bcast over b; other X[:, s_b] at col c0+b;
                result -> mt[:, s_out]."""
                xa = X[0:rows, 0, 0, 2 : 2 + WA]
                anchor = ap_of(
                    xa, SROW * s_a, [[0, nb], [WT, C], [1, WA]]
                )
                xb = X[0:rows, 0, 0, c0 : c0 + WA]
                other = ap_of(xb, SROW * s_b, [[1, nb], [WT, C], [1, WA]])
                d = tp.tile([rows, nb, C, WA], BF16, tag=f"d_{tag}")
                nc.vector.tensor_sub(d, anchor, other)
                g = tp.tile([rows, nb, C, WA], BF16, tag=f"g_{tag}")
                nc.scalar.activation(g, d, AF.Derivative_Erf, scale=SQRT2)
                g01 = tp.tile([rows, nb, WA], BF16, tag=f"g01_{tag}")
                nc.vector.tensor_mul(g01, g[:, :, 0, :], g[:, :, 1, :])
                nc.vector.tensor_mul(mt[0:rows, s_out], g01, g[:, :, 2, :])

            cls("m1", P, 0, 1, 5, 0, M12, 0)    # a=1: X0 vs X1, b=-2..2
            cls("m2", P, 0, 2, 5, 0, M12, 1)    # a=2: X0 vs X2 (row127 junk)
            cls("e0lo", P, 0, 0, 2, 3, ME, 0)   # a=0 rows 0..127
            cls("e0hi", P, 1, 1, 2, 3, ME, 1)   # a=0 rows 1..128

            # ---- stage C ------------------------------------------------
            SM = WA  # b-plane stride of M tiles (elements)
            SC = 5 * WA  # s-plane stride of M12

            def m12_ap(s0, ns, k0, nk, col0, dcol):
                base = M12[0:P, 0, 0, 0:W]
                return ap_of(
                    base,
                    SC * s0 + SM * k0 + col0,
                    [[SC, ns], [SM + dcol, nk], [1, W]],
                )

            # D combos: plane t = sum of b-planes t..t+2 at col +0
            T4D = tp.tile([P, 2, 4, W], BF16, tag="t4d")
            nc.vector.tensor_add(
                T4D, m12_ap(0, 2, 0, 4, 2, 0), m12_ap(0, 2, 1, 4, 2, 0)
            )
            D12 = tp.tile([P, 2, 3, W], BF16, tag="d12")
            nc.vector.tensor_add(D12, T4D[:, :, 0:3, :], m12_ap(0, 2, 2, 3, 2, 0))

            # U combos: plane t = sum_{j=t..t+2} M[:, :, j, col + 4 - j]
            T4U = tp.tile([P, 2, 4, W], BF16, tag="t4u")
            nc.vector.tensor_add(
                T4U, m12_ap(0, 2, 0, 4, 4, -1), m12_ap(0, 2, 1, 4, 3, -1)
            )
            U12 = tp.tile([P, 2, 3, W], BF16, tag="u12")
            nc.vector.tensor_add(U12, T4U[:, :, 0:3, :], m12_ap(0, 2, 2, 3, 2, -1))

            # A combos (same-row sums + k), merged over (lo, hi):
            #  nc=0: E01(j) + E02(j); nc=1: E01(j-1) + E01(j); nc=2: E02(j-2)+E01(j-1)
            SE = 2 * WA  # s-plane stride of ME

            def me_ap(k, col0):
                base = ME[0:P, 0, 0, 0:W]
                return ap_of(base, SM * k + col0, [[SE, 2], [1, W]])

            A = tp.tile([P, 2, 3, W], BF16, tag="a")
            pairs = [((0, 2), (1, 2)), ((0, 1), (0, 2)), ((1, 0), (0, 1))]
            for ncol, ((k0, o0), (k1, o1)) in enumerate(pairs):
                nc.vector.scalar_tensor_tensor(
                    out=A[:, :, ncol, :],
                    in0=me_ap(k0, o0),
                    scalar=K,
                    in1=me_ap(k1, o1),
                    op0=AOP.add,
                    op1=AOP.add,
                )

            # Y = Ahi + U1  (so S1 = Y + D1h, S2 = Y@+1 + U2)
            Y = tp.tile([P, 3, W], BF16, tag="y")
            nc.vector.tensor_add(Y, A[:, 1, :, :], U12[:, 0, :, :])

            # partition-shifted copies via SBUF-to-SBUF DMA
            D1h = tp.tile([POUT, 3, W], BF16, tag="psd1")
            nc.gpsimd.dma_start(out=D1h, in_=D12[1 : 1 + POUT, 0, :, :])
            Yh = tp.tile([POUT, 3, W], BF16, tag="psy")
            nc.gpsimd.dma_start(out=Yh, in_=Y[1 : 1 + POUT, :, :])

            # ---- role sums [127, 3(nc), 254] ----------------------------
            def diag(t, s_sel, plane0, dplane, splane):
                """[POUT, 3, WOUT] view: plane i at (plane0 + i*dplane,
                col + i); splane = s-slice stride offset or None for 3d tiles."""
                if splane is None:
                    base = t[0:POUT, 0, 0:WOUT]
                    stride = W
                else:
                    base = t[0:POUT, s_sel, 0, 0:WOUT]
                    stride = W
                return dataclasses.replace(
                    _with_dims(base, [[dplane * stride + 1, 3], [1, WOUT]]),
                    offset=base.offset + plane0 * stride,
                )

            S0 = tp.tile([POUT, 3, WOUT], BF16, tag="s0")
            nc.vector.tensor_add(S0, diag(A, 0, 0, 1, 0), diag(D12, 0, 2, -1, 0))
            nc.vector.tensor_add(S0, S0, diag(D12, 1, 2, -1, 0))
            S1 = tp.tile([POUT, 3, WOUT], BF16, tag="s1")
            nc.vector.tensor_add(S1, diag(Y, 0, 0, 1, None), diag(D1h, 0, 2, -1, None))
            S2 = tp.tile([POUT, 3, WOUT], BF16, tag="s2")
            nc.vector.tensor_add(S2, diag(Yh, 0, 0, 1, None), diag(U12, 1, 0, 1, 0))

            # ---- stage D: product, log, affine --------------------------
            T0 = tp.tile([POUT, 3, WOUT], BF16, tag="t0")
            nc.vector.tensor_mul(T0, S0, S1)
            T1 = tp.tile([POUT, 3, WOUT], BF16, tag="t1")
            nc.vector.tensor_mul(T1, T0, S2)
            R = tp.tile([POUT, WOUT], BF16, tag="r")
            nc.vector.tensor_mul(R, T1[:, 0, :], T1[:, 1, :])
            PP = tp.tile([POUT, WOUT], BF16, tag="pp")
            nc.vector.tensor_mul(PP, R, T1[:, 2, :])
            L = tp.tile([POUT, WOUT], F32, tag="ln")
            nc.scalar.activation(L, PP, AF.Ln)
            OUT = tp.tile([POUT, WOUT], F32, tag="out")
            nc.vector.tensor_scalar(
                out=OUT,
                in0=L,
                scalar1=-1.0 / 9.0,
                scalar2=AFFINE_C,
                op0=AOP.mult,
                op1=AOP.add,
            )
            # split the output across the two HWDGE queues
            nc.sync.dma_start(out=yout[0:64, :], in_=OUT[0:64, :])
            nc.scalar.dma_start(out=yout[64:POUT, :], in_=OUT[64:POUT, :])
    if not nc.is_finalized():
        nc.finalize()
    return nc


_PROGRAM = None


def _get_program():
    global _PROGRAM
    if _PROGRAM is None:
        _PROGRAM = _build_program()
    return _PROGRAM


def _shard_inputs(x):
    x = np.asarray(x, dtype=np.float32)
    xp = np.zeros((B, 257, C, WT), dtype=ml_dtypes.bfloat16)
    xp[:, :256, :, PAD : PAD + W] = np.transpose(x, (0, 2, 1, 3))
    in_maps = []
    for core in range(8):
        b, half = divmod(core, 2)
        r0 = half * POUT
        in_maps.append({"xin": np.ascontiguousarray(xp[b, r0 : r0 + ROWS_IN + 1])})
    return in_maps


def _gather(results):
    out = np.empty((B, 254, 254), dtype=np.float32)
    for core in range(8):
        b, half = divmod(core, 2)
        out[b, half * POUT : half * POUT + POUT, :] = results[core]["yout"]
    return out


def kernel(x, **_unused):
    nc = _get_program()
    res = run_bass_kernel_spmd(nc, _shard_inputs(x), core_ids=list(range(8)))
    return _gather(res.results)


def kernel_traced(x):
    """Same as kernel() but returns (output, BassKernelResults) with trace."""
    nc = _get_program()
    res = run_bass_kernel_spmd(
        nc, _shard_inputs(x), core_ids=list(range(8)), trace=True
    )
    return _gather(res.results), res
